# revision 89
# baseline (speedup 1.0000x reference)
"""NeuralSDE (ReversibleHeun) Trainium2 kernel, v5 (two-stream pipelined,
drift-deferred, e0-pipelined).

v5 schedule changes over v4 (2120465 -> 2045990 ns):
  - DIFF_FIRST: each m1 emits the diffusion branch (w0f matmul -> silu ->
    8 chunk matmuls+tanhs) first and defers the whole drift branch
    (w0d -> silu -> w1d -> tanh) until after the chunk stream.  The drift
    result F1 is only consumed by the other stream's m2 hook half a step
    later, so it has slack; the diffusion silu gates the chunk cascade,
    which sits on the step's critical cycle (tanh7 -> e0 einsum -> pa ->
    yhat -> aug -> w0 -> silu -> cascade -> tanh7').
  - E0SPLIT+E0RSPLIT: the e0 products for step t+1 are emitted inside
    step t's chunk-tanh stream (halves after tanh3/tanh7 on GPSIMD), and
    the chunks-0-3 partial reduce runs right after the e1 products at c7
    (DVE).  At the next step's head only reduce(4:8) + one add remain,
    cutting the e0 path on the critical cycle from ~6.4us (full Pool
    product + full DVE reduce) to ~1.5us.


Data-parallel over batch: B=4096 split across 8 NeuronCores (512 each).
Within a core the batch is split into TWO streams of 256 that run half a
step out of phase, so each engine's idle time in one stream's dependency
chain is filled by the other stream's work.

On-chip layout is transposed (feature-on-partition, batch-on-free).

Math notes (dt == 1 since ts = arange):
  yhat1   = 2 y - yhat + f + e0            e0 = einsum('bhn,bn->bh', g,  dw)
  Ytilde1 = 2 y1 = Ytilde + F + e0 + F1 + e1   (the yhat_old terms cancel)
With state tile S = [Ytilde(64); F(64)] and a PSUM accumulator
  pa  = sel@e0sum + [I;I]@S_prev            (= Ytilde+F+e0)
  yhat1 = pa - yhat_old                     (DVE, reads pa)
  pa += [0;I]@S_new[64:] + sel@e1sum        (PSUM re-accumulation)
  Ytilde1 = copy(pa)                        (DVE, PSUM->SBUF)
(TensorTensor can read only one PSUM input, so pa+pb is folded into the
PE's PSUM accumulator instead of a DVE add.)

Einsum layout: diffusion features permuted so chunk c row k holds
original feature (h, n) = (k % 64, 2c + k//64).  g/dw/product tiles are
[128, batch, chunk] (chunk innermost) so the 8-chunk sum is a single
DVE tensor_reduce(axis=X); one half-fold matmul sel[k, m] = (m == k%64)
finishes the reduction.  dwx[k, b, c] = dw[b, 2c + k//64] is
materialized per step by two partition-broadcast DMAs (prefetched one
step ahead).  e0 products run on GPSIMD (off the critical path), e1
products on DVE (latency critical, issued as chunk tanhs complete).
Stream A additionally sums e0 with a GPSIMD add tree instead of the DVE
reduce to balance engine load.

lipswish(x) = 0.909*silu(x): the 0.909 is folded into w1 host-side.
The per-step time value rides row 64 of the aug tile via gpsimd.memset
(ts = arange, so the value is the literal step index).
All recurrence-path matmuls stay fp32: the recurrence amplifies
per-step noise ~8%/step, so any low-precision pass in the state path
blows past the fp32 envelope.  The readout matmul is outside the
recurrence and runs in float32r (4x PE throughput).
"""

import os
import sys

sys.path.insert(0, "/opt/trn_rl_repo")

import numpy as np

import concourse.bass as bass
import concourse.mybir as mybir
import concourse.tile as tile
from concourse import bacc
from concourse.bass_utils import run_bass_kernel_spmd

# Problem sizes (hardcoded per spec)
B, T, H, N, IN, W, D = 4096, 128, 64, 16, 32, 128, 8
NCORES = 8
BL = B // NCORES          # 512 batch per core
BS = BL // 2              # 256 batch per stream
NSTEPS = T - 1            # 127
NCHUNK = (H * N) // 128   # 8 chunks of 128 features

F32 = mybir.dt.float32
F32R = mybir.dt.float32r
AF = mybir.ActivationFunctionType
OP = mybir.AluOpType
AX = mybir.AxisListType

STEPS = int(os.environ.get("SDE_STEPS", str(NSTEPS)))
RO_F32R = os.environ.get("SDE_ROF32R", "0") == "1"
# e0 reduction style per stream: "ptree" = gpsimd add tree, "dred" = DVE reduce
E0RED = (os.environ.get("SDE_E0RED_A", "dred"),
         os.environ.get("SDE_E0RED_B", "dred"))
E1STYLE = os.environ.get("SDE_E1STYLE", "half")        # pair_split|pair|half
YCOPY = os.environ.get("SDE_YCOPY", "act")             # act|dve|pool
FLUSH_K = int(os.environ.get("SDE_FLUSH_K", "8"))      # readout DMA cadence
# e0 product route per stream: pmul = gpsimd mul, dmul = DVE mul
E0MUL = (os.environ.get("SDE_E0MUL_A", "pmul"),
         os.environ.get("SDE_E0MUL_B", "pmul"))
# e1 reduction per stream: dred = DVE reduce, ptree = gpsimd tree
E1RED = (os.environ.get("SDE_E1RED_A", "dred"),
         os.environ.get("SDE_E1RED_B", "dred"))
# defer stream-A pb matmuls into the next iteration's PE queue
DEFER_M2A = os.environ.get("SDE_DEFER_M2A", "1") == "1"
# where the deferred readout matmul sits in the PE stream: -1 = before
# w1d, k>=0 = after chunk k
RO_POS = int(os.environ.get("SDE_RO_POS", "-1"))
# chunk index where the other stream's pa-matmuls hook into this m1
# (per-stream: _A applies inside m1(0,...), _B inside m1(1,...))
HOOK2_C = int(os.environ.get("SDE_HOOK2_C", "7"))
HOOK2_CS = (int(os.environ.get("SDE_HOOK2_A", str(HOOK2_C))),
            int(os.environ.get("SDE_HOOK2_B", str(HOOK2_C))))
M2HOOK_C = int(os.environ.get("SDE_M2HOOK_C", "1"))
# emission slot for the e0 first-half product (data ready at c3; later
# slots change the Pool queue interleaving)
E0H1C = int(os.environ.get("SDE_E0H1C", "3"))
# e1 product boundary (E1STYLE=half): early mul covers 0:E1BND at
# tanh_{E1BND-1}, tail mul E1BND:8 at tanh7.  6/7 trade a bigger early
# op for a smaller post-tanh7 tail and one fewer op-overhead
E1BND = int(os.environ.get("SDE_E1BND", "4"))
# stream-B e1 reduce as a GPSIMD tree emitted half a step deferred
# (between m1(B) and m1(A)) so it lands in Pool's idle window instead of
# between the two streams' urgent e0 tail products
E1TDEF = os.environ.get("SDE_E1TDEF", "0") == "1"
# readout deferral distance (1 or 2 steps)
RO_LAG = int(os.environ.get("SDE_RO_LAG", "2"))
# emit both streams' readouts at the seam after stream A's chunk stream
RO_SEAM = os.environ.get("SDE_RO_SEAM", "0") == "1"
# e1 reduce split: 0 = single reduce, 1 = per-half reduces + add
E1SPLIT2 = os.environ.get("SDE_E1SPLIT2", "0") == "1"
# where e0_sum(B) is emitted: 0 = before m1(A), 1 = hooked into m1(A) @ c1
E0B_POS = os.environ.get("SDE_E0B_POS", "0")
# emit e0 products for step t+1 inside step t's chunk-tanh stream (halves
# at tanh3/tanh7) so only the reduce remains at the next iteration's head
E0SPLIT = os.environ.get("SDE_E0SPLIT", "1") == "1"
# with E0SPLIT: also reduce chunks 0-3 inside the tanh stream (right after
# e1mul-p2 at c7), leaving only reduce(4:8) + add at the next step's head
E0RSPLIT = os.environ.get("SDE_E0RSPLIT", "1") == "1"
# engine for the chunks-0-3 e0 partial: dve = one tensor_reduce, pool =
# two-level gpsimd add tree (keeps the partial off the DVE entirely)
E0RSENG = os.environ.get("SDE_E0RSENG", "dve")
# e0 products in quarters at tanh1/3/5/7 (instead of halves at 3/7); the
# partial then covers chunks 0-5 and the head reduce only 6:8
E0QUART = os.environ.get("SDE_E0QUART", "0") == "1"
# e0 products in thirds at tanh3/5/7 (0:4, 4:6, 6:8); partial covers 0:6
# at c7, head reduce only 6:8
E0THIRDS = os.environ.get("SDE_E0THIRDS", "0") == "1"
# e0 product boundary: early product covers chunks 0:E0BND (emitted at
# tanh_{E0BND-1}), tail product E0BND:8 at tanh7.  4 = halves (default);
# 6/7 shrink the post-tanh7 tail (smaller tail mul + head reduce).
E0BND = int(os.environ.get("SDE_E0BND", "4"))
# emit ONE full 8-chunk e0 reduce at the end of the c7 block instead of
# partial+head-reduce+add (less DVE work, but the queued reduce waits on
# the Pool tail product)
E0FULL = os.environ.get("SDE_E0FULL", "0") == "1"
# fold the t-row and bias-row of the w0 matmuls into per-step silu bias
# columns (t is a compile-time constant): drops the aug t-row memset from
# the yhat -> w0 head and the ones-row init DMAs
TBIAS = os.environ.get("SDE_TBIAS", "0") == "1"
# write the e0 partial into a 9th slot of tmp0 so the head becomes a
# single reduce(4:9) instead of reduce(4:8) + add
R0S9 = os.environ.get("SDE_R0S9", "0") == "1"
# mirror of E0RSPLIT for the e1 reduce: partial(0:4) after e1mul-p1 at
# c3, head does reduce(4:8) + add
E1RSPLIT = os.environ.get("SDE_E1RSPLIT", "0") == "1"
E1RSENG = os.environ.get("SDE_E1RSENG", "dve")
# engine for the e1 chunks-0-3 product at c3 (off the critical tail)
E1P1_ENG = os.environ.get("SDE_E1P1_ENG", "dve")
# fold the e0 partial+tail sums into pa via two sel matmuls (skips the
# DVE add of r0p + r0q)
R0PA = os.environ.get("SDE_R0PA", "0") == "1"
# fold the yhat subtraction into the pa PSUM group via a -I matmul and
# copy yhat1 out on the Act engine instead of the DVE sub
YHATPE = os.environ.get("SDE_YHATPE", "0") == "1"
# emit the diffusion branch (pf matmul + silu + chunks) before the drift
# branch, deferring pd/w1d/drift-tanh until after the chunk stream — the
# drift result is not needed until the other stream's m2 hook
DIFF_FIRST = os.environ.get("SDE_DIFF_FIRST", "1") == "1"
# with DIFF_FIRST: 0 = drift branch at end of drift_diff, 1 = after the
# e1 reduce in m1
DRIFT_POS = int(os.environ.get("SDE_DRIFT_POS", "0"))
# split silu into two ops + per-stream chunk PSUM pools (frees p1 banks)
SILU_SPLIT = os.environ.get("SDE_SILU_SPLIT", "1") == "1"
# engine for the yhat1 = pa - yhat_old subtract, per stream
YHAT_ENG = (os.environ.get("SDE_YHAT_A", "dve"),
            os.environ.get("SDE_YHAT_B", "dve"))
# h1p placement: hook = inside other stream's m1 @ HOOK2_C, plain = between
H1P_HOOK = os.environ.get("SDE_H1P_HOOK", "1") == "1"
# split each dwx half-DMA by stream (4 quarter DMAs, stream-A parts first)
DWX_QUARTER = os.environ.get("SDE_DWX_QUARTER", "1") == "1"
H2_BUFS = int(os.environ.get("SDE_H2_BUFS", "2"))
AUG_BUFS = int(os.environ.get("SDE_AUG_BUFS", "3"))
CHUNK_BUFS = int(os.environ.get("SDE_CHUNK_BUFS", "2"))
# 3-pass f32r decomposition of the diffusion w1 matmul.  Host splits the
# weights w = w_hi + w_lo with w_hi carrying 13 explicit mantissa bits
# (FP22-exact) and w_lo the 10-bit remainder; the device splits the silu
# output h the same way.  hi@h_hi + lo@h_hi + hi@h_err reproduces the
# fp32 product to within fp32 rounding (the dropped lo@err term is
# ~2^-26 relative) while each f32r pass runs at 4x the fp32 rate.
W1F3P = os.environ.get("SDE_W1F3P", "0") == "1"
# engine for the h2f mantissa split (AND-mask + subtract)
SPLIT_ENG = os.environ.get("SDE_SPLIT_ENG", "dve")
MANT_MASK = 0xFFFFFC00  # keep 13 explicit mantissa bits
# diagnostic: skip the he pass + split ops (numerically wrong, timing only)
W1F3P_FAKE = os.environ.get("SDE_W1F3P_FAKE", "0") == "1"

_cached = {}


def _enable_jax_cache():
    try:
        import jax
        jax.config.update("jax_compilation_cache_dir", "/root/jaxcache")
        jax.config.update("jax_persistent_cache_min_compile_time_secs", 0.0)
        jax.config.update("jax_persistent_cache_min_entry_size_bytes", -1)
    except Exception:
        pass


def build_module(nsteps=STEPS):
    nc = bacc.Bacc("TRN2", target_bir_lowering=False, debug=False)

    def din(name, shape):
        return nc.dram_tensor(name, list(shape), F32, kind="ExternalInput").ap()

    # per-core data
    noiseaug = din("noiseaug", [IN + 1, BL])            # [33,512] row32 = 1.0
    dwh = din("dwh", [NSTEPS, 2, BL * NCHUNK])          # [t, h, b*8+c] = dW[b,t,2c+h]
    ones_row = din("ones_row", [1, BL])
    # weights (replicated)
    w0aug_drift = din("w0aug_drift", [H + 2, W])        # [66,128]: y(64), t, bias
    w0aug_diff = din("w0aug_diff", [H + 2, W])
    w1s_drift = din("w1s_drift", [W, H])                # 0.909-folded
    w1s_diff = din("w1s_diff", [W, H * N])              # 0.909-folded + phi-permuted
    if W1F3P:
        w1s_diff_lo = din("w1s_diff_lo", [W, H * N])    # low-mantissa remainder
        maskcol = nc.dram_tensor("maskcol", [128, 1], mybir.dt.uint32,
                                 kind="ExternalInput").ap()
    if TBIAS:
        tb_drift = din("tb_drift", [W, T])   # [:, t] = t*w0[64,:] + w0[65,:]
        tb_diff = din("tb_diff", [W, T])
    b1_drift = din("b1_drift", [H, 1])
    b1_diff = din("b1_diff", [128, NCHUNK])             # chunk c bias on [:,c]
    selmat = din("selmat", [128, H])                    # sel[k,m] = (m == k%64)
    ident2 = din("ident2", [128, H])                    # [I;I]
    identlo = din("identlo", [128, H])                  # [0;I]
    if YHATPE:
        negid = din("negid", [H, H])                    # -I
    initw0aug = din("initw0aug", [IN + 1, W])           # [33,128] bias-folded
    initw1 = din("initw1", [W, H])
    initb1 = din("initb1", [H, 1])
    initb1x2 = din("initb1x2", [H, 1])
    ro_w_half = din("ro_w_half", [H, D])                # 0.5*ro_w
    ro_b = din("ro_b", [D, 1])

    outT = nc.dram_tensor("outT", [T, D, BL], F32, kind="ExternalOutput").ap()

    with tile.TileContext(nc) as tc:
        with tc.tile_pool(name="consts", bufs=1) as cp, \
             tc.tile_pool(name="work", bufs=2) as wp, \
             tc.tile_pool(name="psum", bufs=1, space="PSUM") as pp:

            def load_const(name, ap_dram, shape):
                t_ = cp.tile(list(shape), F32, tag=name, name=name)
                nc.sync.dma_start(out=t_[:], in_=ap_dram)
                return t_

            c_w0d = load_const("c_w0d", w0aug_drift, [H + 2, W])
            c_w0f = load_const("c_w0f", w0aug_diff, [H + 2, W])
            c_w1d = load_const("c_w1d", w1s_drift, [W, H])
            c_w1f = load_const("c_w1f", w1s_diff, [W, H * N])
            if W1F3P:
                c_w1fl = load_const("c_w1fl", w1s_diff_lo, [W, H * N])
                c_mask = cp.tile([128, 1], mybir.dt.uint32, tag="c_mask",
                                 name="c_mask")
                nc.sync.dma_start(out=c_mask[:], in_=maskcol)
            if TBIAS:
                c_tbd = load_const("c_tbd", tb_drift, [W, T])
                c_tbf = load_const("c_tbf", tb_diff, [W, T])
            c_b1d = load_const("c_b1d", b1_drift, [H, 1])
            c_b1f = load_const("c_b1f", b1_diff, [128, NCHUNK])
            c_sel = load_const("c_sel", selmat, [128, H])
            c_id2 = load_const("c_id2", ident2, [128, H])
            c_idlo = load_const("c_idlo", identlo, [128, H])
            c_nid = load_const("c_nid", negid, [H, H]) if YHATPE else None
            c_iw0 = load_const("c_iw0", initw0aug, [IN + 1, W])
            c_iw1 = load_const("c_iw1", initw1, [W, H])
            c_ib1 = load_const("c_ib1", initb1, [H, 1])
            c_ib2 = load_const("c_ib2", initb1x2, [H, 1])
            c_row = load_const("c_row", ro_w_half, [H, D])
            c_rob = load_const("c_rob", ro_b, [D, 1])
            c_noise = load_const("c_noise", noiseaug, [IN + 1, BL])

            # persistent per-stream state (ping-pong):
            # aug = [yhat(64); t; ones], S = [Ytilde(64); F(64)],
            # g = diffusion [128, b, c] (chunk innermost)
            aug = [[cp.tile([H + 2, BS], F32, tag=f"aug{s}{i}",
                            name=f"aug{s}{i}") for i in range(AUG_BUFS)]
                   for s in range(2)]
            st = [[cp.tile([128, BS], F32, tag=f"st{s}{i}",
                           name=f"st{s}{i}") for i in range(2)]
                  for s in range(2)]
            gst = [[cp.tile([128, BS, NCHUNK], F32, tag=f"g{s}{i}",
                            name=f"g{s}{i}") for i in range(2)]
                   for s in range(2)]
            # shared dw broadcast tile [128, 512, 8] (4-deep prefetch ring)
            dwx = [cp.tile([128, BL, NCHUNK], F32, tag=f"dwx{i}",
                           name=f"dwx{i}") for i in range(4)]
            # einsum product tiles per stream (9th slot holds the e0
            # partial when R0S9)
            tmp0 = [cp.tile([128, BS, NCHUNK + (1 if R0S9 else 0)], F32,
                            tag=f"tmp0{s}", name=f"tmp0{s}")
                    for s in range(2)]
            tmp1 = [cp.tile([128, BS, NCHUNK], F32, tag=f"tmp1{s}",
                            name=f"tmp1{s}") for s in range(2)]
            # e-sums
            r0 = [cp.tile([128, BS], F32, tag=f"r0{s}", name=f"r0{s}")
                  for s in range(2)]
            r1 = [cp.tile([128, BS], F32, tag=f"r1{s}", name=f"r1{s}")
                  for s in range(2)]
            if E1STYLE == "pair_split" or E1SPLIT2:
                r1p = [cp.tile([128, BS], F32, tag=f"r1p{s}",
                               name=f"r1p{s}") for s in range(2)]
                rt = [cp.tile([128, BS], F32, tag=f"rt{s}", name=f"rt{s}")
                      for s in range(2)]
            if E0RSPLIT:
                r0p = [cp.tile([128, BS], F32, tag=f"r0p{s}",
                               name=f"r0p{s}") for s in range(2)]
                r0q = [cp.tile([128, BS], F32, tag=f"r0q{s}",
                               name=f"r0q{s}") for s in range(2)]
                tr0a = [cp.tile([128, BS, 2], F32, tag=f"tr0a{s}",
                                name=f"tr0a{s}") for s in range(2)]
            if E1RSPLIT:
                r1ps = [cp.tile([128, BS], F32, tag=f"r1ps{s}",
                                name=f"r1ps{s}") for s in range(2)]
                r1qs = [cp.tile([128, BS], F32, tag=f"r1qs{s}",
                                name=f"r1qs{s}") for s in range(2)]
                tr1a = [cp.tile([128, BS, 2], F32, tag=f"tr1a{s}",
                                name=f"tr1a{s}") for s in range(2)]
            # gpsimd tree intermediates (allocated only where routed)
            def tree_tiles(pfx):
                a = cp.tile([128, BS, NCHUNK // 2], F32, tag=f"{pfx}1",
                            name=f"{pfx}1")
                b = cp.tile([128, BS, NCHUNK // 4], F32, tag=f"{pfx}2",
                            name=f"{pfx}2")
                return a, b
            tr = [tree_tiles(f"tr{s}") if E0RED[s] in ("ptree", "psplit")
                  else None for s in range(2)]
            sr = [tree_tiles(f"sr{s}")
                  if (E1RED[s] == "ptree" or (E1TDEF and s == 1)) else None
                  for s in range(2)]
            # readout staging ring: 8 steps per output DMA
            oring = [cp.tile([D, 8, BS], F32, tag=f"or{s}", name=f"or{s}")
                     for s in range(2)]
            # PSUM: 8 banks. Per stream: p1 (1), pa accumulator (1, own
            # bank: start=True resets interact at bank granularity), pepr
            # (1: w1d-out slot 0 / readout slot 1), shared chunk pool (2).
            p1 = None if SILU_SPLIT else \
                 [pp.tile([128, 2, BS], F32, tag=f"p1{s}", name=f"p1{s}")
                  for s in range(2)]

            def chunk_tile(s):
                tag = f"chunk{s}" if SILU_SPLIT else "chunk"
                return pp.tile([128, BS], F32, tag=tag, bufs=CHUNK_BUFS,
                               name=f"pc{s}" if SILU_SPLIT else "pc")
            pab = [pp.tile([H, BS], F32, tag=f"pab{s}", name=f"pab{s}")
                   for s in range(2)]
            pepr = [pp.tile([H, 2, BS], F32, tag=f"pepr{s}", name=f"pepr{s}")
                    for s in range(2)]

            if not TBIAS:
                for s in range(2):
                    for i in range(AUG_BUFS):
                        nc.sync.dma_start(
                            out=aug[s][i][H + 1:H + 2, :],
                            in_=ones_row[0:1, s * BS:(s + 1) * BS])

            def mm(out_ps, lhsT, rhs, start, stop):
                nc.tensor.matmul(out_ps, lhsT, rhs, start=start, stop=stop)

            def load_dwx(t):
                """Fill dwx[(t-1)%4] with dw for step t via bcast DMAs."""
                dst = dwx[(t - 1) % 4]
                base = dwh[t - 1]      # [2, BL*NCHUNK]
                if DWX_QUARTER:
                    # stream-A quarters first so A's products can start
                    # after half the transfer
                    for s in range(2):
                        for half in range(2):
                            src = bass.AP(
                                tensor=base.tensor,
                                offset=(base.offset + half * BL * NCHUNK
                                        + s * BS * NCHUNK),
                                ap=[[0, 64], [1, BS * NCHUNK]])
                            nc.sync.dma_start(
                                out=dst[half * 64:(half + 1) * 64,
                                        s * BS:(s + 1) * BS, :]
                                .rearrange("p b c -> p (b c)"),
                                in_=src)
                else:
                    for half in range(2):
                        src = bass.AP(
                            tensor=base.tensor,
                            offset=base.offset + half * BL * NCHUNK,
                            ap=[[0, 64], [1, BL * NCHUNK]])
                        nc.sync.dma_start(
                            out=dst[half * 64:(half + 1) * 64, :, :]
                            .rearrange("p b c -> p (b c)"),
                            in_=src)
                return dst

            def sslice(ap128, s):
                return ap128[:, s * BS:(s + 1) * BS, :]

            def drift_diff(s, aug_t, f_dst, g_dst, dwx_t=None, tmp_dst=None,
                           ro_t=None, hook=None, hook2=None, dwx_next=None,
                           tcol=0):
                """f_dst[64,BS] = tanh-drift, g_dst[128,BS,8] = tanh-diffusion.
                If dwx_t is given, also issue e1 products into tmp_dst as
                chunk tanhs complete (after c=3 and c=7)."""
                h2 = wp.tile([128, 2, BS], F32, tag=f"h2{s}", bufs=H2_BUFS,
                             name=f"h2{s}")
                if W1F3P:
                    # hi (slot 0) / err (slot 1) mantissa split of h2f
                    h2s = wp.tile([128, 2, BS], F32, tag=f"h2s{s}",
                                  bufs=H2_BUFS, name=f"h2s{s}")
                if DIFF_FIRST:
                    pf = chunk_tile(s)
                    if TBIAS:
                        mm(pf[:], c_w0f[0:H, :], aug_t[0:H, :], start=True,
                           stop=True)
                    else:
                        mm(pf[:], c_w0f[:], aug_t[:], start=True, stop=True)
                    if ro_t is not None and RO_POS < 0:
                        readout(s, ro_t)
                    if TBIAS:
                        nc.scalar.activation(h2[:, 1, :], pf[:], AF.Silu,
                                             bias=c_tbf[:, tcol:tcol + 1])
                    else:
                        nc.scalar.activation(h2[:, 1, :], pf[:], AF.Silu)
                elif SILU_SPLIT:
                    pd = chunk_tile(s)
                    mm(pd[:], c_w0d[:], aug_t[:], start=True, stop=True)
                    pf = chunk_tile(s)
                    mm(pf[:], c_w0f[:], aug_t[:], start=True, stop=True)
                    if ro_t is not None and RO_POS < 0:
                        readout(s, ro_t)
                    nc.scalar.activation(h2[:, 0, :], pd[:], AF.Silu)
                    nc.scalar.activation(h2[:, 1, :], pf[:], AF.Silu)
                else:
                    mm(p1[s][:, 0, :], c_w0d[:], aug_t[:], start=True,
                       stop=True)
                    mm(p1[s][:, 1, :], c_w0f[:], aug_t[:], start=True,
                       stop=True)
                    if ro_t is not None and RO_POS < 0:
                        readout(s, ro_t)
                    nc.scalar.activation(h2[:], p1[s][:], AF.Silu)
                if W1F3P and not W1F3P_FAKE:
                    seng = nc.gpsimd if SPLIT_ENG == "pool" else nc.vector
                    U32 = mybir.dt.uint32
                    seng.tensor_single_scalar(
                        h2s[:, 0, :].bitcast(U32), h2[:, 1, :].bitcast(U32),
                        c_mask[:], OP.bitwise_and)
                    seng.tensor_sub(h2s[:, 1, :], h2[:, 1, :], h2s[:, 0, :])
                def drift_branch():
                    if DIFF_FIRST:
                        pd = chunk_tile(s)
                        if TBIAS:
                            mm(pd[:], c_w0d[0:H, :], aug_t[0:H, :],
                               start=True, stop=True)
                            nc.scalar.activation(h2[:, 0, :], pd[:], AF.Silu,
                                                 bias=c_tbd[:, tcol:tcol + 1])
                        else:
                            mm(pd[:], c_w0d[:], aug_t[:], start=True,
                               stop=True)
                            nc.scalar.activation(h2[:, 0, :], pd[:], AF.Silu)
                    mm(pepr[s][:, 0, :], c_w1d[:], h2[:, 0, :], start=True,
                       stop=True)
                    nc.scalar.activation(f_dst, pepr[s][:, 0, :], AF.Tanh,
                                         bias=c_b1d[:])
                if not DIFF_FIRST:
                    drift_branch()
                for c in range(NCHUNK):
                    pc = chunk_tile(s)
                    sl = slice(c * 128, (c + 1) * 128)
                    if W1F3P and W1F3P_FAKE:
                        hr = h2[:, 1, :].bitcast(F32R)
                        mm(pc[:], c_w1f[:, sl].bitcast(F32R), hr,
                           start=True, stop=False)
                        mm(pc[:], c_w1fl[:, sl].bitcast(F32R), hr,
                           start=False, stop=True)
                    elif W1F3P:
                        hh = h2s[:, 0, :].bitcast(F32R)
                        he = h2s[:, 1, :].bitcast(F32R)
                        mm(pc[:], c_w1f[:, sl].bitcast(F32R), hh,
                           start=True, stop=False)
                        mm(pc[:], c_w1fl[:, sl].bitcast(F32R), hh,
                           start=False, stop=False)
                        mm(pc[:], c_w1f[:, sl].bitcast(F32R), he,
                           start=False, stop=True)
                    else:
                        mm(pc[:], c_w1f[:, sl], h2[:, 1, :],
                           start=True, stop=True)
                    nc.scalar.activation(g_dst[:, :, c], pc[:], AF.Tanh,
                                         bias=c_b1f[:, c:c + 1])
                    if c == RO_POS and ro_t is not None:
                        readout(s, ro_t)
                    if E0THIRDS:
                        e0cs = {3: (0, 4), 5: (4, 6), 7: (6, 8)}
                    elif E0QUART:
                        e0cs = {c_: (c_ - 1, c_ + 1) for c_ in (1, 3, 5, 7)}
                    else:
                        e0cs = {max(E0H1C, E0BND - 1): (0, E0BND),
                                7: (E0BND, 8)}
                    if dwx_next is not None and c in e0cs:
                        lo, hi = e0cs[c]
                        meng = nc.gpsimd if E0MUL[s] == "pmul" else nc.vector
                        meng.tensor_mul(
                            tmp0[s][:, :, lo:hi], g_dst[:, :, lo:hi],
                            sslice(dwx_next, s)[:, :, lo:hi])
                        if c == 5 and E0QUART and E0RSPLIT:
                            nc.vector.tensor_reduce(
                                r0p[s][:], tmp0[s][:, :, 0:6], AX.X, OP.add)

                    if c == M2HOOK_C and hook is not None:
                        hook()
                    if c == HOOK2_CS[s] and hook2 is not None:
                        hook2()
                    if dwx_t is not None:
                        e1cs = {E1BND - 1: (0, E1BND), 7: (E1BND, 8)}
                        if E1STYLE == "half" and c in e1cs:
                            lo, hi = e1cs[c]
                            e1eng = (nc.gpsimd if (c == E1BND - 1 and
                                     E1P1_ENG == "pool") else nc.vector)
                            e1eng.tensor_mul(
                                tmp_dst[:, :, lo:hi], g_dst[:, :, lo:hi],
                                sslice(dwx_t, s)[:, :, lo:hi])
                            if c == 3 and E1SPLIT2:
                                nc.vector.tensor_reduce(
                                    r1p[s][:], tmp_dst[:, :, 0:4], AX.X,
                                    OP.add)
                        elif E1STYLE != "half" and c % 2 == 1:
                            lo, hi = c - 1, c + 1
                            nc.vector.tensor_mul(
                                tmp_dst[:, :, lo:hi], g_dst[:, :, lo:hi],
                                sslice(dwx_t, s)[:, :, lo:hi])
                            if c == 5 and E1STYLE == "pair_split":
                                # early partial sum of chunks 0-5 shortens
                                # the post-tanh7 tail to two small adds
                                nc.vector.tensor_reduce(
                                    r1p[s][:], tmp_dst[:, :, 0:6], AX.X,
                                    OP.add)
                    if (dwx_t is not None and c == 3 and E1RSPLIT):
                        # partial e1 reduce (chunks 0-3), off the tail
                        if E1RSENG == "pool":
                            nc.gpsimd.tensor_add(tr1a[s][:],
                                                 tmp_dst[:, :, 0:2],
                                                 tmp_dst[:, :, 2:4])
                            nc.gpsimd.tensor_add(r1ps[s][:],
                                                 tr1a[s][:, :, 0],
                                                 tr1a[s][:, :, 1])
                        else:
                            nc.vector.tensor_reduce(
                                r1ps[s][:], tmp_dst[:, :, 0:4], AX.X,
                                OP.add)
                    if (c == 7 and dwx_next is not None and E0RSPLIT
                            and not E0QUART):
                        # partial e0 reduce (chunks 0-3, or 0-5 with
                        # thirds) for step t+1, after the e1 products so
                        # it stays off this step's critical DVE sequence
                        if E0RSENG == "pool":
                            nc.gpsimd.tensor_add(tr0a[s][:],
                                                 tmp0[s][:, :, 0:2],
                                                 tmp0[s][:, :, 2:4])
                            nc.gpsimd.tensor_add(r0p[s][:],
                                                 tr0a[s][:, :, 0],
                                                 tr0a[s][:, :, 1])
                        elif E0FULL:
                            nc.vector.tensor_reduce(
                                r0[s][:], tmp0[s][:], AX.X, OP.add)
                        else:
                            phi_ = 6 if E0THIRDS else E0BND
                            pdst = (tmp0[s][:, :, NCHUNK] if R0S9
                                    else r0p[s][:])
                            nc.vector.tensor_reduce(
                                pdst, tmp0[s][:, :, 0:phi_], AX.X,
                                OP.add)
                if DIFF_FIRST:
                    if DRIFT_POS == 0:
                        drift_branch()
                    else:
                        return drift_branch

            def e0_sum(s, t):
                """r0[s] = sum_c gst_prev * dwx (products may already be
                in tmp0 via the e0split hooks)."""
                prev = (t + 1) % 2
                dwx_t = dwx[(t - 1) % 4]
                if not E0SPLIT:
                    meng = nc.gpsimd if E0MUL[s] == "pmul" else nc.vector
                    meng.tensor_mul(tmp0[s][:], gst[s][prev][:],
                                    sslice(dwx_t, s))
                if E0SPLIT and E0RSPLIT:
                    if E0FULL:
                        return  # full reduce already emitted at c7
                    qlo = 6 if (E0QUART or E0THIRDS) else E0BND
                    if R0S9:
                        # partial lives in slot 8: one fused reduce(4:9)
                        nc.vector.tensor_reduce(
                            r0[s][:], tmp0[s][:, :, qlo:NCHUNK + 1], AX.X,
                            OP.add)
                        return
                    nc.vector.tensor_reduce(r0q[s][:], tmp0[s][:, :, qlo:8],
                                            AX.X, OP.add)
                    if not R0PA:
                        nc.vector.tensor_add(r0[s][:], r0p[s][:], r0q[s][:])
                elif E0RED[s] == "ptree":
                    a_, b_ = tr[s]
                    nc.gpsimd.tensor_add(a_[:], tmp0[s][:, :, 0:4],
                                         tmp0[s][:, :, 4:8])
                    nc.gpsimd.tensor_add(b_[:], a_[:, :, 0:2],
                                         a_[:, :, 2:4])
                    nc.gpsimd.tensor_add(r0[s][:], b_[:, :, 0],
                                         b_[:, :, 1])
                elif E0RED[s] == "psplit":
                    a_, _ = tr[s]
                    nc.gpsimd.tensor_add(a_[:], tmp0[s][:, :, 0:4],
                                         tmp0[s][:, :, 4:8])
                    nc.vector.tensor_reduce(r0[s][:], a_[:], AX.X, OP.add)
                else:
                    nc.vector.tensor_reduce(r0[s][:], tmp0[s][:], AX.X,
                                            OP.add)

            def readout(s, t):
                """Stage step t's readout in the ring; flush 8 at a time."""
                yt_ap = st[s][t % 2][0:H, :]
                if RO_F32R:
                    mm(pepr[s][0:D, 1, :], c_row[:].bitcast(F32R),
                       yt_ap.bitcast(F32R), start=True, stop=True)
                else:
                    mm(pepr[s][0:D, 1, :], c_row[:], yt_ap, start=True,
                       stop=True)
                ring = oring[s]
                nc.scalar.activation(ring[:, t % 8, :], pepr[s][0:D, 1, :],
                                     AF.Identity, bias=c_rob[:])
                if t % FLUSH_K == FLUSH_K - 1 or t == nsteps:
                    k = t % FLUSH_K + 1
                    t0 = t - k + 1
                    dst = bass.AP(
                        tensor=outT.tensor,
                        offset=outT.offset + t0 * D * BL + s * BS,
                        ap=[[BL, D], [D * BL, k], [1, BS]])
                    # Act-queue DMA: dep (ring) was written just above
                    nc.scalar.dma_start(out=dst, in_=ring[:, 0:k, :])

            def h1p(s, t):
                """pa matmuls, yhat1, t-row (issued after e0_sum's work)."""
                prev, new = (t - 1) % AUG_BUFS, t % AUG_BUFS
                sprev = (t + 1) % 2
                if R0PA and E0SPLIT and E0RSPLIT:
                    mm(pab[s][:], c_sel[:], r0p[s][:], start=True,
                       stop=False)
                    mm(pab[s][:], c_sel[:], r0q[s][:], start=False,
                       stop=False)
                else:
                    mm(pab[s][:], c_sel[:], r0[s][:], start=True, stop=False)
                if YHATPE:
                    mm(pab[s][:], c_id2[:], st[s][sprev][:], start=False,
                       stop=False)
                    mm(pab[s][:], c_nid[:], aug[s][prev][0:H, :],
                       start=False, stop=True)
                    nc.scalar.copy(aug[s][new][0:H, :], pab[s][:])
                else:
                    mm(pab[s][:], c_id2[:], st[s][sprev][:], start=False,
                       stop=True)
                    # yhat1 = pa - yhat_old ; row64 = t
                    yeng = nc.gpsimd if YHAT_ENG[s] == "pool" else nc.vector
                    yeng.tensor_sub(aug[s][new][0:H, :], pab[s][:],
                                    aug[s][prev][0:H, :])
                if not TBIAS:
                    nc.gpsimd.memset(aug[s][new][H:H + 1, :], float(t))

            def m1(s, t, hook=None, hook2=None):
                """MLP evals + e1 products/reduce; also flushes the deferred
                readout of step t-RO_LAG (its Ytilde is long since ready)."""
                new = t % 2
                if s == 0 and t + 1 < nsteps:
                    load_dwx(t + 2)
                dwx_t = dwx[(t - 1) % 4]
                ro_t = t - RO_LAG if t > RO_LAG else None
                db = drift_diff(s, aug[s][t % AUG_BUFS],
                                st[s][new][H:128, :], gst[s][new],
                                dwx_t=dwx_t, tmp_dst=tmp1[s],
                                ro_t=None if RO_SEAM else ro_t,
                                hook=hook, hook2=hook2,
                                dwx_next=dwx[t % 4]
                                if (E0SPLIT and t < nsteps) else None,
                                tcol=t)
                if RO_SEAM and ro_t is not None:
                    # both streams' readouts ride the seam after stream A's
                    # chunk stream, where the PE otherwise starves waiting
                    # for yhat(B) -> aug(B)
                    if s == 0:
                        readout(0, ro_t)
                        readout(1, ro_t)
                if E1TDEF and s == 1:
                    pass  # reduce emitted deferred via e1tree()
                elif E1RSPLIT:
                    nc.vector.tensor_reduce(r1qs[s][:], tmp1[s][:, :, 4:8],
                                            AX.X, OP.add)
                    nc.vector.tensor_add(r1[s][:], r1ps[s][:], r1qs[s][:])
                elif E1SPLIT2:
                    nc.vector.tensor_reduce(rt[s][:], tmp1[s][:, :, 4:8],
                                            AX.X, OP.add)
                    nc.vector.tensor_add(r1[s][:], rt[s][:], r1p[s][:])
                elif E1RED[s] == "ptree":
                    a_, b_ = sr[s]
                    nc.gpsimd.tensor_add(a_[:], tmp1[s][:, :, 0:4],
                                         tmp1[s][:, :, 4:8])
                    nc.gpsimd.tensor_add(b_[:], a_[:, :, 0:2],
                                         a_[:, :, 2:4])
                    nc.gpsimd.tensor_add(r1[s][:], b_[:, :, 0],
                                         b_[:, :, 1])
                elif E1STYLE == "pair_split":
                    nc.vector.tensor_add(rt[s][:], tmp1[s][:, :, 6],
                                         tmp1[s][:, :, 7])
                    nc.vector.tensor_add(r1[s][:], rt[s][:], r1p[s][:])
                else:
                    nc.vector.tensor_reduce(r1[s][:], tmp1[s][:], AX.X,
                                            OP.add)
                if callable(db):
                    db()

            def e1tree(s):
                a_, b_ = sr[s]
                nc.gpsimd.tensor_add(a_[:], tmp1[s][:, :, 0:4],
                                     tmp1[s][:, :, 4:8])
                nc.gpsimd.tensor_add(b_[:], a_[:, :, 0:2], a_[:, :, 2:4])
                nc.gpsimd.tensor_add(r1[s][:], b_[:, :, 0], b_[:, :, 1])

            def m2(s, t):
                """pb accumulation onto pa, Ytilde1 copy-out."""
                new = t % 2
                nc.tensor.matmul(pab[s][:], c_idlo[H:128, :],
                                 st[s][new][H:128, :], start=False,
                                 stop=False, skip_group_check=True)
                nc.tensor.matmul(pab[s][:], c_sel[:], r1[s][:],
                                 start=False, stop=True,
                                 skip_group_check=True)
                if YHATPE:
                    # pab holds pa - yhat_old (the -I pass); re-add
                    # yhat_old so Ytilde1 = pa_full
                    nc.gpsimd.tensor_add(st[s][new][0:H, :], pab[s][:],
                                         aug[s][(t - 1) % AUG_BUFS][0:H, :])
                    return
                # Ytilde1 = copy(pa) PSUM->SBUF
                if YCOPY == "act":
                    nc.scalar.copy(st[s][new][0:H, :], pab[s][:])
                elif YCOPY == "pool":
                    nc.gpsimd.tensor_copy(st[s][new][0:H, :], pab[s][:])
                else:
                    nc.vector.tensor_scalar_mul(st[s][new][0:H, :],
                                                pab[s][:], 1.0)

            # ---- init: y0 = relu(noise@w0+b0)@w1 + b1, per stream ----
            for s in range(2):
                p0 = chunk_tile(s)
                mm(p0[:], c_iw0[:], c_noise[:, s * BS:(s + 1) * BS],
                   start=True, stop=True)
                h0 = wp.tile([128, BS], F32, tag=f"h0{s}", name=f"h0{s}")
                nc.scalar.activation(h0[:], p0[:], AF.Relu)
                mm(pepr[s][:, 0, :], c_iw1[:], h0[:], start=True, stop=True)
                nc.scalar.activation(aug[s][0][0:H, :], pepr[s][:, 0, :],
                                     AF.Identity, bias=c_ib1[:])
                nc.scalar.activation(st[s][0][0:H, :], pepr[s][:, 0, :],
                                     AF.Identity, bias=c_ib2[:], scale=2.0)
                if not TBIAS:
                    nc.gpsimd.memset(aug[s][0][H:H + 1, :], 0.0)
                if s == 0:
                    load_dwx(1)
                    load_dwx(2)
                db0 = drift_diff(s, aug[s][0], st[s][0][H:128, :], gst[s][0],
                                 dwx_next=dwx[0] if E0SPLIT else None,
                                 tcol=0)
                if callable(db0):
                    db0()
                readout(s, 0)

            # ---- time loop: A leads, B lags half a step ----
            # Emission order packs each engine's in-order queue so a queue
            # head's dependencies are produced ~a block earlier:
            #   e0v(A,t)  M1(B,t-1)  paA  M1(A,t)  M2(B,t-1)  e0v(B,t)
            #   M2(A,t)  paB
            for t in range(1, nsteps + 1):
                e0_sum(0, t)
                hk2a = (lambda tn=t: h1p(0, tn)) if H1P_HOOK else None
                hk2b = (lambda tn=t: h1p(1, tn)) if H1P_HOOK else None
                if t >= 2:
                    m1(1, t - 1,
                       hook=lambda tm=t - 1: m2(0, tm),
                       hook2=hk2a)
                if E1TDEF and t >= 2:
                    e1tree(1)
                if not H1P_HOOK or t < 2:
                    h1p(0, t)
                if E0B_POS == "0":
                    e0_sum(1, t)
                    m1(0, t,
                       hook=(lambda tm=t - 1: m2(1, tm)) if t >= 2 else None,
                       hook2=hk2b)
                else:
                    def _hkA(tm=t):
                        e0_sum(1, tm)
                        if tm >= 2:
                            m2(1, tm - 1)
                    m1(0, t, hook=_hkA, hook2=hk2b)
                if not H1P_HOOK:
                    h1p(1, t)
            m1(1, nsteps, hook=lambda: m2(0, nsteps))
            if E1TDEF:
                e1tree(1)
            m2(1, nsteps)
            for tt in range(max(1, nsteps - RO_LAG + 1), nsteps + 1):
                readout(0, tt)
                readout(1, tt)

    nc.compile()
    return nc


def prep_inputs(ts, init_noise, dW,
                init_w0, init_b0, init_w1, init_b1,
                drift_w0, drift_b0, drift_w1, drift_b1,
                diff_w0, diff_b0, diff_w1, diff_b1,
                ro_w, ro_b):
    f32 = np.float32

    def aug_w(w0, b0):
        # reorder: rows 0-63 = y coeffs, row 64 = t coeff, row 65 = bias
        return np.vstack([w0[1:], w0[0:1], b0[None, :]]).astype(f32)

    w0aug_drift = aug_w(np.asarray(drift_w0), np.asarray(drift_b0))
    w0aug_diff = aug_w(np.asarray(diff_w0), np.asarray(diff_b0))
    w1s_drift = (0.909 * np.asarray(drift_w1)).astype(f32)

    # phi permutation: chunk c, row k <- original feature (k%64)*16 + 2c + k//64
    k = np.arange(128)
    phi = np.empty(128 * NCHUNK, np.int64)
    for c in range(NCHUNK):
        phi[c * 128 + k] = (k % 64) * 16 + 2 * c + k // 64
    w1s_diff = (0.909 * np.asarray(diff_w1))[:, phi].astype(f32)
    if W1F3P:
        bits = np.ascontiguousarray(w1s_diff).view(np.uint32)
        w1s_hi = (bits & np.uint32(MANT_MASK)).view(f32)
        w1s_lo = (w1s_diff - w1s_hi).astype(f32)
        w1s_diff = w1s_hi
    b1_diff = np.asarray(diff_b1, f32)[phi].reshape(NCHUNK, 128).T.copy()

    b1_drift = np.asarray(drift_b1, f32).reshape(H, 1)
    sel = np.zeros((128, H), f32)
    sel[k, k % 64] = 1.0
    id2 = np.concatenate([np.eye(H, dtype=f32), np.eye(H, dtype=f32)], 0)
    idlo = np.concatenate([np.zeros((H, H), f32), np.eye(H, dtype=f32)], 0)
    nid = (-np.eye(H)).astype(f32)
    initw0aug = np.vstack([np.asarray(init_w0), np.asarray(init_b0)[None, :]]
                          ).astype(f32)
    initw1 = np.asarray(init_w1, f32)
    initb1 = np.asarray(init_b1, f32).reshape(H, 1)
    ro_w_half = (0.5 * np.asarray(ro_w)).astype(f32)
    rob = np.asarray(ro_b, f32).reshape(D, 1)
    ones = np.ones((1, BL), f32)

    if YHATPE:
        shared_negid = {"negid": nid}
    else:
        shared_negid = {}
    shared = dict(
        w0aug_drift=w0aug_drift, w0aug_diff=w0aug_diff,
        w1s_drift=w1s_drift, w1s_diff=w1s_diff,
        b1_drift=b1_drift, b1_diff=b1_diff, selmat=sel,
        ident2=id2, identlo=idlo,
        initw0aug=initw0aug, initw1=initw1, initb1=initb1,
        initb1x2=(2.0 * initb1), ro_w_half=ro_w_half, ro_b=rob,
        ones_row=ones,
    )
    shared.update(shared_negid)
    if W1F3P:
        shared["w1s_diff_lo"] = w1s_lo
        shared["maskcol"] = np.full((128, 1), MANT_MASK, np.uint32)
    if TBIAS:
        tcols = np.arange(T, dtype=f32)[None, :]
        shared["tb_drift"] = (w0aug_drift[64][:, None] * tcols
                              + w0aug_drift[65][:, None]).astype(f32)
        shared["tb_diff"] = (w0aug_diff[64][:, None] * tcols
                             + w0aug_diff[65][:, None]).astype(f32)

    init_noise = np.asarray(init_noise, f32)
    dW = np.asarray(dW, f32)
    in_maps = []
    for core in range(NCORES):
        b0_, b1_ = core * BL, (core + 1) * BL
        na = np.concatenate([init_noise[b0_:b1_].T, np.ones((1, BL), f32)], 0)
        # dwh[t, h, b*8 + c] = dW[b0_+b, t, 2c+h]
        d = dW[b0_:b1_].transpose(1, 0, 2)                  # [127, BL, 16]
        d = d.reshape(NSTEPS, BL, NCHUNK, 2)                # [t, b, c, h]
        dwht = np.ascontiguousarray(
            d.transpose(0, 3, 1, 2).reshape(NSTEPS, 2, BL * NCHUNK))
        m = dict(shared)
        m["noiseaug"] = np.ascontiguousarray(na)
        m["dwh"] = dwht
        in_maps.append(m)
    return in_maps


def kernel(**inputs) -> np.ndarray:
    _enable_jax_cache()
    key = ("nc", STEPS)
    if key not in _cached:
        _cached[key] = build_module(STEPS)
    nc = _cached[key]
    in_maps = prep_inputs(**inputs)
    res = run_bass_kernel_spmd(nc, in_maps, core_ids=list(range(NCORES)))
    outs = []
    for c in range(NCORES):
        o = res.results[c]["outT"]          # [T, D, BL]
        outs.append(np.ascontiguousarray(o.transpose(2, 0, 1)))  # [BL, T, D]
    return np.concatenate(outs, axis=0).astype(np.float32)

